# revision 1
# baseline (speedup 1.0000x reference)
"""Deformable convolution (B=4, C=256, 64x64, COUT=256, 3x3) on 8 trn2 NeuronCores.

Sharding: data-parallel over (batch, output-row-half): core i handles batch i//2,
output rows [32*(i%2), 32*(i%2)+32). Weight replicated.

Device pipeline per core:
  1. index/fraction math from offsets (DVE, fp32, immediate-scalar ops only)
  2. one dma_gather per (tap, n-chunk) from a host-staged "quad" image Q in
     DRAM: Q[y*68+x] = [P[y,x,:], P[y+1,x,:]] (fp16, zero-padded borders), so
     each 2KB gathered element carries the full 2x2 bilinear patch
     [TL BL TR BR] for 256 channels.
  3. bilinear combine on DVE: one broadcast tensor_tensor multiply
     (weights [128,(k j q)] broadcast along c) + 3 pairwise adds
  4. PE transposes columns to [C,N]-major, ACT copies PSUM->SBUF
  5. fp16 GEMM (K=2304) accumulating in PSUM, fp32 output
"""

import os
import sys

for _p in ("/root/.axon_site", "/root/.axon_site/_ro/trn_rl_repo", "/opt/trn_rl_repo"):
    if os.path.isdir(_p) and _p not in sys.path:
        sys.path.append(_p)

import numpy as np

import concourse.bass as bass
import concourse.bacc as bacc
import concourse.mybir as mybir
from concourse.tile import TileContext

# ---------------------------------------------------------------- constants
B, CIN, H, W = 4, 256, 64, 64
COUT, KH, KW = 256, 3, 3
KK = KH * KW                      # 9 taps
HO = WO = 64
HOH = 32                          # output rows per core
N = HOH * WO                      # 2048 positions per core
NJ = 16                           # 128-blocks of N
NCH = 2                           # gather chunks (h)
NJH = NJ // NCH                   # j' blocks per chunk = 8
NIDX = NJH * 128                  # 1024 indices per gather
PAD = 2                           # zero-pad border of the staged image
Hp = Wp = H + 2 * PAD             # 68
NROW = Hp * Wp                    # 4624 quad rows
QE = 4 * CIN                      # 1024 elements per gathered quad
KB = 2 * KK                       # 18 K-blocks of 128
C288 = KK * 2 * NJ                # 288
C144 = KK * NJ                    # 144
FP16 = mybir.dt.float16
FP32 = mybir.dt.float32
I16 = mybir.dt.int16
I32 = mybir.dt.int32
OP = mybir.AluOpType

_MAX_WAITS = 1


def _split_multiwait_instructions(nc):
    """This walrus build rejects >1 sync wait on one instruction ('Too many
    sync wait commands'); hoist extras onto single-wait EventSemaphore
    instructions inserted just before it."""
    fn = nc.m.functions[0]
    for bb in fn.blocks:
        new_insts = []
        for inst in bb.instructions:
            si = getattr(inst, "sync_info", None)
            if si is not None and si.on_wait and len(si.on_wait) > _MAX_WAITS:
                waits = list(si.on_wait)
                for k, w in enumerate(waits[_MAX_WAITS:]):
                    ev = mybir.InstEventSemaphore(
                        name=f"{inst.name}_wsplit{k}",
                        ins=[],
                        outs=[],
                        sync_info=mybir.SyncInfo(on_wait=[w], on_update=[]),
                    )
                    ev.engine = inst.engine
                    new_insts.append(ev)
                si.on_wait = waits[:_MAX_WAITS]
            new_insts.append(inst)
        bb.instructions[:] = new_insts


# ---------------------------------------------------------------- device kernel
def build_nc(split_waits=True):
    nc = bacc.Bacc()
    img = nc.dram_tensor("img", [NROW, QE // 2], FP16, kind="ExternalInput")
    offg = nc.dram_tensor("offg", [128, C288], FP32, kind="ExternalInput")
    grid = nc.dram_tensor("grid", [128, C288], FP32, kind="ExternalInput")
    wT = nc.dram_tensor("wT", [KB * 128, COUT], FP16, kind="ExternalInput")
    ident = nc.dram_tensor("ident", [128, 128], FP16, kind="ExternalInput")
    out = nc.dram_tensor("out", [COUT, N], FP32, kind="ExternalOutput")

    # gather source: rows of 2*QE fp16 with stride QE (overlapping x-pairs)
    img_src = bass.AP(img[:].tensor, 0, [[QE // 2, NROW - 1], [1, QE]])

    with TileContext(nc) as tc:
        with (
            tc.tile_pool(name="const", bufs=1) as constp,
            tc.tile_pool(name="small", bufs=1) as smallp,
            tc.tile_pool(name="gath", bufs=3) as gathp,
            tc.tile_pool(name="prod", bufs=2) as prodp,
            tc.tile_pool(name="interp", bufs=3) as vp,
            tc.tile_pool(name="cols", bufs=3) as colsp,
            tc.tile_pool(name="osb", bufs=2) as osbp,
            tc.tile_pool(name="pt", bufs=3, space="PSUM") as ptp,
            tc.tile_pool(name="pout", bufs=1, space="PSUM") as poutp,
        ):
            # ---- constants
            id_sb = constp.tile([128, 128], FP16)
            nc.sync.dma_start(id_sb[:], ident[:])
            w_sb = constp.tile([128, KB, COUT], FP16)
            nc.sync.dma_start(w_sb[:], wT[:].rearrange("(kb p) o -> p kb o", p=128))
            offg_sb = constp.tile([128, C288], FP32)
            nc.sync.dma_start(offg_sb[:], offg[:])
            grid_sb = constp.tile([128, C288], FP32)
            nc.sync.dma_start(grid_sb[:], grid[:])

            # ---- stage A: sampling positions, fractions, weights, indices
            pp = smallp.tile([128, C288], FP32, tag="pp")
            nc.vector.tensor_tensor(out=pp[:], in0=offg_sb[:], in1=grid_sb[:],
                                    op=OP.add)
            # floor(pp): int-cast rounds-to-nearest on HW but truncates in
            # CoreSim; correct either to floor via (cast > pp) ? cast-1 : cast.
            p_i = smallp.tile([128, C288], I32, tag="pi")
            nc.vector.tensor_copy(out=p_i[:], in_=pp[:])
            p_f = smallp.tile([128, C288], FP32, tag="pf")
            nc.vector.tensor_copy(out=p_f[:], in_=p_i[:])
            gt_t = smallp.tile([128, C288], FP32, tag="gtt")
            nc.vector.tensor_tensor(out=gt_t[:], in0=p_f[:], in1=pp[:],
                                    op=OP.is_gt)
            nc.vector.tensor_tensor(out=p_f[:], in0=p_f[:], in1=gt_t[:],
                                    op=OP.subtract)
            fr = smallp.tile([128, C288], FP32, tag="fr")
            nc.vector.tensor_tensor(out=fr[:], in0=pp[:], in1=p_f[:],
                                    op=OP.subtract)
            omfr = smallp.tile([128, C288], FP32, tag="omfr")
            nc.vector.tensor_scalar(out=omfr[:], in0=fr[:], scalar1=-1.0,
                                    scalar2=1.0, op0=OP.mult, op1=OP.add)

            # per-tap (k, d, j) views: y = d0, x = d1 -> [128, 9, 16]
            def yx(t):
                v4 = t[:].rearrange("p (k d j) -> p k d j", d=2, j=NJ)
                return v4[:, :, 0, :], v4[:, :, 1, :]

            fr_y, fr_x = yx(fr)
            om_y, om_x = yx(omfr)
            pf_y, pf_x = yx(p_f)

            # bilinear weights -> w4 [128, (k j q)] fp16, q order (TL,BL,TR,BR)
            w4 = smallp.tile([128, C144 * 4], FP16, tag="w4")
            w4v = w4[:].rearrange("p (k j q) -> p k j q", k=KK, j=NJ)
            nc.vector.tensor_tensor(out=w4v[:, :, :, 0], in0=om_y, in1=om_x,
                                    op=OP.mult)  # TL: (1-ly)(1-lx)
            nc.vector.tensor_tensor(out=w4v[:, :, :, 1], in0=fr_y, in1=om_x,
                                    op=OP.mult)  # BL: ly(1-lx)
            nc.vector.tensor_tensor(out=w4v[:, :, :, 2], in0=om_y, in1=fr_x,
                                    op=OP.mult)  # TR: (1-ly)lx
            nc.vector.tensor_tensor(out=w4v[:, :, :, 3], in0=fr_y, in1=fr_x,
                                    op=OP.mult)  # BR: ly lx

            # indices (y0,x0 carry a +16 bias from the host grid):
            # idx = clamp(y0-14,0,67)*68 + clamp(x0-14,0,66)
            tt_ = smallp.tile([128, C144], FP32, tag="tt")
            ss_ = smallp.tile([128, C144], FP32, tag="ss")
            t3 = tt_[:].rearrange("p (k j) -> p k j", j=NJ)
            s3 = ss_[:].rearrange("p (k j) -> p k j", j=NJ)
            nc.vector.tensor_scalar(out=t3, in0=pf_y, scalar1=-14.0,
                                    scalar2=0.0, op0=OP.add, op1=OP.max)
            nc.vector.tensor_scalar(out=tt_[:], in0=tt_[:], scalar1=67.0,
                                    scalar2=float(Wp), op0=OP.min, op1=OP.mult)
            nc.vector.tensor_scalar(out=s3, in0=pf_x, scalar1=-14.0,
                                    scalar2=0.0, op0=OP.add, op1=OP.max)
            nc.vector.tensor_scalar(out=ss_[:], in0=ss_[:], scalar1=66.0,
                                    scalar2=None, op0=OP.min)
            idxf = smallp.tile([128, C144], FP32, tag="idxf")
            nc.vector.tensor_tensor(out=idxf[:], in0=tt_[:], in1=ss_[:],
                                    op=OP.add)
            idxs = smallp.tile([128, C144], I16, tag="idxs")
            nc.vector.tensor_copy(out=idxs[:], in_=idxf[:])

            # fold [128, 144] -> 16 partitions (idx i at [i%16, i//16]),
            # then replicate to all 8 partition groups (Q7 cores).
            idx16 = constp.tile([128, 8 * C144], I16)
            idx16v = idx16[:].rearrange("p (a b) -> p a b", b=8)
            for r in range(8):
                nc.sync.dma_start(out=idx16v[0:16, :, r],
                                  in_=idxs[r * 16:(r + 1) * 16, :])
            for g in range(1, 8):
                nc.sync.dma_start(out=idx16[g * 16:(g + 1) * 16, :],
                                  in_=idx16[0:16, :])

            # ---- stages B-E per (h, k)
            for h in range(NCH):
                pout = [poutp.tile([128, NIDX], FP32, tag=f"pout{ob}",
                                   name=f"pout{ob}_{h}")
                        for ob in range(2)]
                for k in range(KK):
                    g = gathp.tile([128, NJH, QE], FP16, tag="g")
                    base = (k * NJ + h * NJH) * 8
                    nc.gpsimd.dma_gather(
                        g[:], img_src, idx16[:, base:base + NIDX // 16],
                        NIDX, NIDX, QE, elem_step=QE // 2)

                    # products: one broadcast TT multiply over the whole tile
                    prods = prodp.tile([128, NJH, 4, CIN], FP16, tag="prods")
                    wv = w4[:, (k * NJ + h * NJH) * 4:]
                    wb = bass.AP(wv.tensor, wv.offset,
                                 [wv.ap[0], [4, NJH], [1, 4], [0, CIN]])
                    nc.vector.tensor_tensor(
                        out=prods[:],
                        in0=g[:].rearrange("p a (q c) -> p a q c", q=4),
                        in1=wb, op=OP.mult)
                    # v = (TL'+BL') + (TR'+BR')
                    v = vp.tile([128, NJH, CIN], FP16, tag="v")
                    vb = vp.tile([128, NJH, CIN], FP16, tag="vb")
                    nc.vector.tensor_tensor(out=v[:], in0=prods[:, :, 0, :],
                                            in1=prods[:, :, 1, :], op=OP.add)
                    nc.vector.tensor_tensor(out=vb[:], in0=prods[:, :, 2, :],
                                            in1=prods[:, :, 3, :], op=OP.add)
                    nc.vector.tensor_tensor(out=v[:], in0=v[:], in1=vb[:],
                                            op=OP.add)

                    for cb in range(2):
                        pt = ptp.tile([128, NJH, 128], FP16, tag="pt")
                        for j in range(NJH):
                            nc.tensor.transpose(
                                pt[:, j, :], v[:, j, cb * 128:(cb + 1) * 128],
                                id_sb[:])
                        cols = colsp.tile([128, NIDX], FP16, tag="cols")
                        nc.scalar.copy(out=cols[:],
                                       in_=pt[:].rearrange("p a b -> p (a b)"))
                        kb = k * 2 + cb
                        for ob in range(2):
                            for ns in range(2):
                                nc.tensor.matmul(
                                    pout[ob][:, ns * 512:(ns + 1) * 512],
                                    lhsT=w_sb[:, kb, ob * 128:(ob + 1) * 128],
                                    rhs=cols[:, ns * 512:(ns + 1) * 512],
                                    start=(kb == 0), stop=(kb == KB - 1))

                for ob in range(2):
                    osb = osbp.tile([128, NIDX], FP32, tag="osb")
                    nc.scalar.copy(out=osb[:], in_=pout[ob][:])
                    nc.sync.dma_start(
                        out=out[ob * 128:(ob + 1) * 128,
                                h * NIDX:(h + 1) * NIDX],
                        in_=osb[:])

    nc.compile()
    if split_waits:
        _split_multiwait_instructions(nc)
    return nc


_NC_CACHE = None


def _get_nc():
    global _NC_CACHE
    if _NC_CACHE is None:
        _NC_CACHE = build_nc()
    return _NC_CACHE


# ---------------------------------------------------------------- host prep
def _prep_core_inputs(x, offset, weight):
    """Build the 8 per-core input maps (pure layout/pad/cast transforms)."""
    x = np.asarray(x, np.float32)
    offset = np.asarray(offset, np.float32)
    weight = np.asarray(weight, np.float32)

    imgs = []
    for b in range(B):
        pimg = np.zeros((Hp + 1, Wp, CIN), np.float16)
        pimg[PAD:PAD + H, PAD:PAD + W, :] = x[b].transpose(1, 2, 0)
        # quad rows: Q[y*68+x] = [P[y,x,:], P[y+1,x,:]]
        quad = np.concatenate([pimg[:Hp], pimg[1:Hp + 1]], axis=2)
        imgs.append(np.ascontiguousarray(quad.reshape(NROW, QE // 2)))

    wT = np.ascontiguousarray(
        weight.transpose(2, 3, 1, 0).reshape(KB * 128, COUT).astype(np.float16))
    ident = np.eye(128, dtype=np.float16)

    # base grid (+16 bias for floor correction): cols (k, d, j), n = j*128+p
    p = np.arange(128)
    j = np.arange(NJ)
    n = j[None, :] * 128 + p[:, None]          # [128, 16]
    grids = []
    for half in range(2):
        ho0 = half * HOH
        g = np.empty((128, KK, 2, NJ), np.float32)
        for kh in range(KH):
            for kw in range(KW):
                k = kh * KW + kw
                g[:, k, 0, :] = kh + (ho0 + n // WO) - 1 + 16
                g[:, k, 1, :] = kw + (n % WO) - 1 + 16
        grids.append(np.ascontiguousarray(g.reshape(128, C288)))

    in_maps = []
    for core in range(8):
        b, half = core // 2, core % 2
        ho0 = half * HOH
        offc = offset[b].reshape(KK, 2, HO, WO)[:, :, ho0:ho0 + HOH, :]
        offc = offc.reshape(KK, 2, NJ, 128)          # [k, d, j, p]
        offg_np = np.ascontiguousarray(
            offc.transpose(3, 0, 1, 2).reshape(128, C288))
        in_maps.append({
            "img": imgs[b],
            "offg": offg_np,
            "grid": grids[half],
            "wT": wT,
            "ident": ident,
        })
    return in_maps


def _assemble(results):
    out = np.empty((B, COUT, HO, WO), np.float32)
    for core, r in enumerate(results):
        b, half = core // 2, core % 2
        out[b, :, half * HOH:(half + 1) * HOH, :] = (
            r["out"].reshape(COUT, HOH, WO))
    return out


def kernel(x, offset, weight):
    from concourse.bass_utils import run_bass_kernel_spmd

    nc = _get_nc()
    in_maps = _prep_core_inputs(x, offset, weight)
    res = run_bass_kernel_spmd(nc, in_maps, core_ids=list(range(8)))
    return _assemble(res.results)



# revision 52
# speedup vs baseline: 2.0885x; 2.0885x over previous
"""Deformable convolution (B=4, C=256, 64x64, COUT=256, 3x3) on 8 trn2 NeuronCores.

Sharding: data-parallel over (batch, output-row-half): core i handles batch i//2,
output rows [32*(i%2), 32*(i%2)+32). Weight replicated.

Device pipeline per core:
  1. index math from offsets on DVE in "A-layout" (partition = n%128); bilinear
     weights in "B-layout" (partition = (j',p') matching the gather output
     permutation), pair-duplicated for DVE 2x mode
  2. idx fold to the 16-partition-wrapped gather layout via 8 DMAs of
     contiguous 16B runs (col order (k,h,r,j') -> gathered partition (j',p'),
     slot r)
  3. one dma_gather per (half, tap) from a host-staged "quad" image Q in DRAM:
     Q[y*68+x] = [P[y,x,:], P[y+1,x,:]] (fp16, zero-padded borders); each 2KB
     element carries the 2x2 bilinear patch [TL BL TR BR] for 256 channels
  4. bilinear combine: ONE 2x-mode multiply (merged (r,q) stride-2 dim) +
     2 adds on DVE; the final pair-sum rides the PE transpose via PSUM
     accumulation (pt = A^T + B^T)
  5. fp16 GEMM (K=2304) accumulating in PSUM, fp32 output; host unpermutes
     the (r,j',p') column order.
"""

import os
import sys

for _p in ("/root/.axon_site", "/root/.axon_site/_ro/trn_rl_repo", "/opt/trn_rl_repo"):
    if os.path.isdir(_p) and _p not in sys.path:
        sys.path.append(_p)

import numpy as np

import concourse.bass as bass
import concourse.bacc as bacc
import concourse.mybir as mybir
from concourse.tile import TileContext

# ---------------------------------------------------------------- constants
B, CIN, H, W = 4, 256, 64, 64
COUT, KH, KW = 256, 3, 3
KK = KH * KW                      # 9 taps
STRIDE, CPAD, DIL = 1, 1, 1
HO = WO = 64
HOH = 32                          # output rows per core
N = HOH * WO                      # 2048 positions per core
NJ = 16                           # 128-blocks of N
NCH = 2                           # gather chunks (h)
NJH = NJ // NCH                   # j' blocks per chunk = 8
NIDX = NJH * 128                  # 1024 indices per gather
PAD = 2                           # zero-pad border of the staged image
Hp = Wp = H + 2 * PAD             # 68
NROW = Hp * Wp                    # 4624 quad rows
QE = 4 * CIN                      # 1024 elements per gathered quad
KB = 2 * KK                       # 18 K-blocks of 128
C288 = KK * 2 * NJ                # 288
C144 = KK * NJ                    # 144
FP16 = mybir.dt.float16
FP32 = mybir.dt.float32
I16 = mybir.dt.int16
I32 = mybir.dt.int32
OP = mybir.AluOpType

_MAX_WAITS = 1
_RELAY_SEM = 150  # declared in ant_sem_names but unreferenced


def _relay_gather1_waits(nc):
    """Move the first real gather's multi-wait set onto the idle PE
    sequencer.  Hoisted single-wait events cost ~1.1us EACH on a sequencer;
    on Pool they serialize after the dummy gather's descriptor gen (~9us
    before gather #1 can start).  On PE they process concurrently with the
    dummy, each bumping a relay semaphore; the gather then carries ONE
    wait (relay >= n)."""
    fn = nc.m.functions[0]
    gathers = [i for bb in fn.blocks for i in bb.instructions
               if type(i).__name__ == "InstDMAGatherAnt"]
    if len(gathers) < 2:
        return
    g0, g1 = gathers[0], gathers[1]

    def relay_update():
        return mybir.SyncUpdate(
            sync_type="semaphore", id=_RELAY_SEM, ant_name="relay_idx16",
            update_mode="sem-add-imm", update_value=1, update_reg=None)

    for bb in fn.blocks:
        insts = bb.instructions
        try:
            gi = insts.index(g1)
        except ValueError:
            continue
        try:
            start = insts.index(g0)
        except ValueError:
            start = 0
        n = 0
        # retarget the framework's Pool event-sems (g1's hoisted deps) to PE
        for inst in insts[start:gi]:
            if (type(inst).__name__ == "InstEventSemaphore"
                    and inst.engine == mybir.EngineType.Pool
                    and inst.sync_info.on_wait
                    and not inst.sync_info.on_update):
                inst.engine = mybir.EngineType.PE
                inst.sync_info.on_update = [relay_update()]
                n += 1
        if n == 0:
            return
        # fold g1's own wait(s) into the relay as PE events too
        events = []
        for k, w in enumerate(list(g1.sync_info.on_wait)):
            ev = mybir.InstEventSemaphore(
                name=f"relay_idx16_{k}", ins=[], outs=[],
                sync_info=mybir.SyncInfo(on_wait=[w],
                                         on_update=[relay_update()]),
            )
            ev.engine = mybir.EngineType.PE
            events.append(ev)
            n += 1
        insts[gi:gi] = events
        g1.sync_info.on_wait = [mybir.SyncWait(
            sync_type="semaphore", id=_RELAY_SEM, ant_name="relay_idx16",
            wait_mode="sem-ge-imm", wait_value=n, wait_reg=None)]
        return


def _split_multiwait_instructions(nc):
    """This walrus build rejects >1 sync wait on one instruction ('Too many
    sync wait commands'); hoist extras onto single-wait EventSemaphore
    instructions inserted just before it."""
    fn = nc.m.functions[0]
    for bb in fn.blocks:
        new_insts = []
        for inst in bb.instructions:
            si = getattr(inst, "sync_info", None)
            if si is not None and si.on_wait and len(si.on_wait) > _MAX_WAITS:
                waits = list(si.on_wait)
                for k, w in enumerate(waits[_MAX_WAITS:]):
                    ev = mybir.InstEventSemaphore(
                        name=f"{inst.name}_wsplit{k}",
                        ins=[],
                        outs=[],
                        sync_info=mybir.SyncInfo(on_wait=[w], on_update=[]),
                    )
                    ev.engine = inst.engine
                    new_insts.append(ev)
                si.on_wait = waits[:_MAX_WAITS]
            new_insts.append(inst)
        bb.instructions[:] = new_insts


# ---------------------------------------------------------------- device kernel
def build_nc(split_waits=True):
    nc = bacc.Bacc(num_swdge_queues=4)
    img = nc.dram_tensor("img", [NROW, QE // 2], FP16, kind="ExternalInput")
    offg = nc.dram_tensor("offg", [128, C288], FP32, kind="ExternalInput")
    grid = nc.dram_tensor("grid", [128, C288], FP32, kind="ExternalInput")
    offgB = nc.dram_tensor("offgB", [128, C288], FP32, kind="ExternalInput")
    gridB = nc.dram_tensor("gridB", [128, C288], FP32, kind="ExternalInput")
    wT = nc.dram_tensor("wT", [128, KB * COUT], FP16, kind="ExternalInput")
    ident = nc.dram_tensor("ident", [128, 128], FP16, kind="ExternalInput")
    dummyidx = nc.dram_tensor("dummyidx", [128, 64], I16, kind="ExternalInput")
    out = nc.dram_tensor("out", [COUT, N], FP32, kind="ExternalOutput")

    # gather source: rows of 2*QE fp16 with stride QE (overlapping x-pairs)
    img_src = bass.AP(img[:].tensor, 0, [[QE // 2, NROW - 1], [1, QE]])

    with TileContext(nc) as tc:
        with (
            tc.tile_pool(name="const", bufs=1) as constp,
            tc.tile_pool(name="small", bufs=1) as smallp,
            tc.tile_pool(name="gath", bufs=5) as gathp,
            tc.tile_pool(name="prod", bufs=2) as prodp,
            tc.tile_pool(name="interp", bufs=2) as vp,
            tc.tile_pool(name="cols", bufs=3) as colsp,
            tc.tile_pool(name="osb", bufs=2) as osbp,
            tc.tile_pool(name="pt", bufs=2, space="PSUM") as ptp,
            tc.tile_pool(name="pout", bufs=1, space="PSUM") as poutp,
        ):
            # ---- constants (offsets/grids first: they gate stage A; weights
            # aren't needed until the first GEMM)
            # all four offset/grid loads on the sync queue (they gate the DVE
            # stream, which the scheduler interleaves A/B); ident/weights on
            # the scalar queue (behind ACT_TABLE_LOAD, needed much later)
            idx0 = constp.tile([128, 64], I16)
            nc.sync.dma_start(idx0[:], dummyidx[:])
            offg_sb = constp.tile([128, C288], FP32)
            nc.sync.dma_start(offg_sb[:], offg[:])
            grid_sb = constp.tile([128, C288], FP32)
            nc.sync.dma_start(grid_sb[:], grid[:])
            offgB_sb = constp.tile([128, C288], FP32)
            nc.sync.dma_start(offgB_sb[:], offgB[:])
            gridB_sb = constp.tile([128, C288], FP32)
            nc.sync.dma_start(gridB_sb[:], gridB[:])
            id_sb = constp.tile([128, 128], FP16)
            nc.scalar.dma_start(id_sb[:], ident[:])
            w_sb = constp.tile([128, KB, COUT], FP16)
            nc.scalar.dma_start(w_sb[:], wT[:].rearrange("p (kb o) -> p kb o", kb=KB))

            # dummy gather: triggers the Q7 gather-library load + shape
            # specialization (~13us + gen) early, overlapped with stage A
            # instead of gating the first real gather.  IDENTICAL parameters
            # to the real gathers (reg-count included: a smaller count
            # de-specializes the ucode and costs +1.4us on EVERY gather);
            # distinct host-provided indices keep its transfer fast, and it
            # sits on queue 0 while gather #1 uses queue 1.
            # reg=128: only 128 valid indices -> gen ~2us and a 256KB transfer
            # (the next gather waits for the previous queue's transfer drain)
            dummy = constp.tile([128, NJH, QE], FP16)
            nc.gpsimd.dma_gather(dummy[:], img_src, idx0[:], NIDX, 128, QE,
                                 elem_step=QE // 2)

            # ---- stage A (idx side, partition = n%128, cols (k, d, (h j')))
            pp = smallp.tile([128, C288], FP32, tag="pp")
            nc.vector.tensor_tensor(out=pp[:], in0=offg_sb[:], in1=grid_sb[:],
                                    op=OP.add)
            # floor(pp): int-cast rounds-to-nearest on HW but truncates in
            # CoreSim; correct either to floor via (cast > pp) ? cast-1 : cast.
            p_i = smallp.tile([128, C288], I32, tag="pi")
            nc.vector.tensor_copy(out=p_i[:], in_=pp[:])
            p_f = smallp.tile([128, C288], FP32, tag="pf")
            nc.vector.tensor_copy(out=p_f[:], in_=p_i[:])
            gt_t = smallp.tile([128, C288], FP32, tag="gtt")
            nc.vector.tensor_tensor(out=gt_t[:], in0=p_f[:], in1=pp[:],
                                    op=OP.is_gt)
            nc.vector.tensor_tensor(out=p_f[:], in0=p_f[:], in1=gt_t[:],
                                    op=OP.subtract)

            pf4 = p_f[:].rearrange("p (k d j) -> p k d j", d=2, j=NJ)
            pf_y, pf_x = pf4[:, :, 0, :], pf4[:, :, 1, :]

            # indices (y0,x0 carry a +16 bias from the host grid):
            # idx = clamp(y0-14,0,67)*68 + clamp(x0-14,0,66)
            tt_ = smallp.tile([128, C144], FP32, tag="tt")
            ss_ = smallp.tile([128, C144], FP32, tag="ss")
            t3 = tt_[:].rearrange("p (k j) -> p k j", j=NJ)
            s3 = ss_[:].rearrange("p (k j) -> p k j", j=NJ)
            nc.vector.tensor_scalar(out=t3, in0=pf_y, scalar1=-14.0,
                                    scalar2=0.0, op0=OP.add, op1=OP.max)
            nc.vector.tensor_scalar(out=tt_[:], in0=tt_[:], scalar1=67.0,
                                    scalar2=float(Wp), op0=OP.min, op1=OP.mult)
            nc.vector.tensor_scalar(out=s3, in0=pf_x, scalar1=-14.0,
                                    scalar2=0.0, op0=OP.add, op1=OP.max)
            nc.vector.tensor_scalar(out=ss_[:], in0=ss_[:], scalar1=66.0,
                                    scalar2=None, op0=OP.min)
            idxf = smallp.tile([128, C144], FP32, tag="idxf")
            nc.vector.tensor_tensor(out=idxf[:], in0=tt_[:], in1=ss_[:],
                                    op=OP.add)
            idxs = smallp.tile([128, C144], I16, tag="idxs")
            nc.vector.tensor_copy(out=idxs[:], in_=idxf[:])

            # ---- idx fold: [128,(k h j')] -> 16-wrapped [16,(k h r j')],
            # contiguous 16B runs (8 x int16), split across two HWDGE queues,
            # then replicate to 8 groups by doubling.  Runs while DVE does
            # stage B below.
            idxs_v = idxs[:].rearrange("p (k h j) -> p k h j", h=NCH, j=NJH)
            idx16 = constp.tile([128, KB * 64], I16)
            idx16v = idx16[:].rearrange("p (k h r j) -> p k h r j", k=KK,
                                        h=NCH, r=8)
            for rr in range(8):
                eng = nc.sync if rr % 2 == 0 else nc.scalar
                eng.dma_start(out=idx16v[0:16, :, :, rr, :],
                              in_=idxs_v[rr * 16:(rr + 1) * 16, :, :, :])
            nc.sync.dma_start(out=idx16[16:32, :], in_=idx16[0:16, :])
            nc.sync.dma_start(out=idx16[32:64, :], in_=idx16[0:32, :])
            nc.sync.dma_start(out=idx16[64:128, :], in_=idx16[0:64, :])

            # ---- stage B (w4 side, partition = (j', p'), cols (k, d, (h r)))
            ppB = smallp.tile([128, C288], FP32, tag="ppB")
            nc.vector.tensor_tensor(out=ppB[:], in0=offgB_sb[:], in1=gridB_sb[:],
                                    op=OP.add)
            p_iB = smallp.tile([128, C288], I32, tag="piB")
            nc.vector.tensor_copy(out=p_iB[:], in_=ppB[:])
            p_fB = smallp.tile([128, C288], FP32, tag="pfB")
            nc.vector.tensor_copy(out=p_fB[:], in_=p_iB[:])
            gt_B = smallp.tile([128, C288], FP32, tag="gtB")
            nc.vector.tensor_tensor(out=gt_B[:], in0=p_fB[:], in1=ppB[:],
                                    op=OP.is_gt)
            nc.vector.tensor_tensor(out=p_fB[:], in0=p_fB[:], in1=gt_B[:],
                                    op=OP.subtract)
            frB = smallp.tile([128, C288], FP32, tag="frB")
            nc.vector.tensor_tensor(out=frB[:], in0=ppB[:], in1=p_fB[:],
                                    op=OP.subtract)
            omB = smallp.tile([128, C288], FP32, tag="omB")
            nc.vector.tensor_scalar(out=omB[:], in0=frB[:], scalar1=-1.0,
                                    scalar2=1.0, op0=OP.mult, op1=OP.add)

            frv = frB[:].rearrange("p (k d m) -> p k d m", d=2, m=NJ)
            omv = omB[:].rearrange("p (k d m) -> p k d m", d=2, m=NJ)
            f_y, f_x = frv[:, :, 0, :], frv[:, :, 1, :]
            o_y, o_x = omv[:, :, 0, :], omv[:, :, 1, :]

            # w4p [128, (k (h r) q pair)] fp16, q order (TL,BL,TR,BR), each
            # value duplicated in a contiguous pair for DVE 2x mode
            w4p = smallp.tile([128, C144 * 4 * 2], FP16, tag="w4p")
            w4pv = w4p[:].rearrange("p (k m q pr) -> p k m q pr", k=KK, m=NJ,
                                    q=4)
            for q, (wa, wb_) in enumerate(
                    [(o_y, o_x), (f_y, o_x), (o_y, f_x), (f_y, f_x)]):
                for pr in range(2):
                    nc.vector.tensor_tensor(out=w4pv[:, :, :, q, pr],
                                            in0=wa, in1=wb_, op=OP.mult)

            # ---- stages B-E per (h, k)
            for h in range(NCH):
                pout = [poutp.tile([128, NIDX], FP32, tag=f"pout{ob}",
                                   name=f"pout{ob}_{h}")
                        for ob in range(2)]
                for k in range(KK):
                    g = gathp.tile([128, NJH, QE], FP16, tag="g")
                    base = (k * 2 + h) * 64
                    nc.gpsimd.dma_gather(
                        g[:], img_src, idx16[:, base:base + 64],
                        NIDX, NIDX, QE, elem_step=QE // 2,
                        queue_num=(1 + h * KK + k) % 2)

                    # products: ONE 2x-mode multiply over the whole tile.
                    # dims: [p][rq merged (stride 256/2 resp.)][cpair][pair]
                    prods = prodp.tile([128, NJH, 4, CIN], FP16, tag="prods")
                    g_ap = bass.AP(g[:].tensor, g[:].offset,
                                   [g[:].ap[0], [256, 32], [2, 128], [1, 2]])
                    p_ap = bass.AP(prods[:].tensor, prods[:].offset,
                                   [prods[:].ap[0], [256, 32], [2, 128],
                                    [1, 2]])
                    wv = w4p[:, (k * 2 + h) * 64:]
                    w_ap = bass.AP(wv.tensor, wv.offset,
                                   [wv.ap[0], [2, 32], [0, 128], [1, 2]])
                    nc.vector.tensor_tensor(out=p_ap, in0=g_ap, in1=w_ap,
                                            op=OP.mult)
                    # one pair-sum on DVE; the other two terms ride the PE
                    # transposes (issued FIRST so the PE starts from raw
                    # prods before the DVE add lands)
                    ab = vp.tile([128, 1, NJH, CIN], FP16, tag="ab")
                    nc.vector.tensor_tensor(out=ab[:, 0], in0=prods[:, :, 0, :],
                                            in1=prods[:, :, 1, :], op=OP.add)

                    for cb in range(2):
                        # transpose + pair-sums on PE: pt = TR^T + BR^T + A^T,
                        # via real matmuls (lhsT^T @ I) accumulating in fp32
                        # PSUM (transpose-mode accumulate diverges on HW)
                        pt = ptp.tile([128, NJH, 128], FP32, tag="pt")
                        for j in range(NJH):
                            cs = slice(cb * 128, (cb + 1) * 128)
                            nc.tensor.matmul(pt[:, j, :],
                                             lhsT=prods[:, j, 2, cs],
                                             rhs=id_sb[:], start=True,
                                             stop=False)
                            nc.tensor.matmul(pt[:, j, :],
                                             lhsT=prods[:, j, 3, cs],
                                             rhs=id_sb[:], start=False,
                                             stop=False)
                            nc.tensor.matmul(pt[:, j, :],
                                             lhsT=ab[:, 0, j, cs],
                                             rhs=id_sb[:], start=False,
                                             stop=True)
                        cols = colsp.tile([128, NIDX], FP16, tag="cols")
                        nc.scalar.copy(out=cols[:],
                                       in_=pt[:].rearrange("p a b -> p (a b)"))
                        kb = k * 2 + cb
                        for ob in range(2):
                            for ns in range(2):
                                nc.tensor.matmul(
                                    pout[ob][:, ns * 512:(ns + 1) * 512],
                                    lhsT=w_sb[:, kb, ob * 128:(ob + 1) * 128],
                                    rhs=cols[:, ns * 512:(ns + 1) * 512],
                                    start=(kb == 0), stop=(kb == KB - 1))

                for ob in range(2):
                    osb = osbp.tile([128, NIDX], FP32, tag="osb")
                    # split the PSUM readout + store in halves across two
                    # HWDGE queues so the final writes drain sooner
                    for nsb in range(2):
                        sl = slice(nsb * 512, (nsb + 1) * 512)
                        nc.scalar.copy(out=osb[:, sl], in_=pout[ob][:, sl])
                        eng = nc.sync if (ob + nsb) % 2 == 0 else nc.scalar
                        eng.dma_start(
                            out=out[ob * 128:(ob + 1) * 128,
                                    h * NIDX + nsb * 512:
                                    h * NIDX + (nsb + 1) * 512],
                            in_=osb[:, sl])

    nc.compile()
    _relay_gather1_waits(nc)
    if split_waits:
        _split_multiwait_instructions(nc)
    return nc


_NC_CACHE = None


def _get_nc():
    global _NC_CACHE
    if _NC_CACHE is None:
        _NC_CACHE = build_nc()
    return _NC_CACHE


# ---------------------------------------------------------------- host prep
def _prep_core_inputs(x, offset, weight):
    """Build the 8 per-core input maps (pure layout/pad/cast transforms)."""
    x = np.asarray(x, np.float32)
    offset = np.asarray(offset, np.float32)
    weight = np.asarray(weight, np.float32)

    imgs = []
    for b in range(B):
        pimg = np.zeros((Hp + 1, Wp, CIN), np.float16)
        pimg[PAD:PAD + H, PAD:PAD + W, :] = x[b].transpose(1, 2, 0)
        # quad rows: Q[y*68+x] = [P[y,x,:], P[y+1,x,:]]
        quad = np.concatenate([pimg[:Hp], pimg[1:Hp + 1]], axis=2)
        imgs.append(np.ascontiguousarray(quad.reshape(NROW, QE // 2)))

    # weight pre-arranged to the SBUF layout [p, kb, cout]
    wk = weight.transpose(2, 3, 1, 0).reshape(KB, 128, COUT)  # [kb, p, o]
    wT = np.ascontiguousarray(
        wk.transpose(1, 0, 2).reshape(128, KB * COUT).astype(np.float16))
    ident = np.eye(128, dtype=np.float16)
    # dummy-gather indices: 128 valid distinct rows up front, -1 padding
    pw = np.arange(16)
    sw = np.arange(64)
    didx = (sw[None, :] * 16 + pw[:, None]).astype(np.int16)   # [16, 64]
    didx[:, 8:] = -1
    dummyidx = np.tile(didx, (8, 1))                            # [128, 64]

    # A-layout grid (+16 bias): partition = n%128, cols (k, d, j); n = j*128+p
    p = np.arange(128)
    j = np.arange(NJ)
    n = j[None, :] * 128 + p[:, None]          # [128, 16]
    grids = []
    for half in range(2):
        ho0 = half * HOH
        ga = np.empty((128, KK, 2, NJ), np.float32)
        for kh in range(KH):
            for kw in range(KW):
                k = kh * KW + kw
                ga[:, k, 0, :] = kh + (ho0 + n // WO) - 1 + 16
                ga[:, k, 1, :] = kw + (n % WO) - 1 + 16
        grids.append(np.ascontiguousarray(ga.reshape(128, C288)))

    # B-layout: partition P = j'*16+p'', cols (k, d, (h r));
    # n = (h*8+j')*128 + r*16 + p''
    jB = np.arange(NJH)
    pB = np.arange(16)
    hB = np.arange(NCH)
    rB = np.arange(8)
    nB = ((hB[None, None, :, None] * NJH + jB[:, None, None, None]) * 128
          + rB[None, None, None, :] * 16 + pB[None, :, None, None])  # [j',p'',h,r]
    nBf = nB.reshape(128, NJ)                  # [(j' p''), (h r)]
    gridsB = []
    for half in range(2):
        ho0 = half * HOH
        gb = np.empty((128, KK, 2, NJ), np.float32)
        for kh in range(KH):
            for kw in range(KW):
                k = kh * KW + kw
                gb[:, k, 0, :] = kh + (ho0 + nBf // WO) - 1 + 16
                gb[:, k, 1, :] = kw + (nBf % WO) - 1 + 16
        gridsB.append(np.ascontiguousarray(gb.reshape(128, C288)))

    in_maps = []
    for core in range(8):
        b, half = core // 2, core % 2
        ho0 = half * HOH
        offc = offset[b].reshape(KK, 2, HO, WO)[:, :, ho0:ho0 + HOH, :]
        offc = offc.reshape(KK, 2, NJ, 128)          # [k, d, j, p]
        offg_np = np.ascontiguousarray(
            offc.transpose(3, 0, 1, 2).reshape(128, C288))
        # B-layout offsets: offc[k, d, n] at [P=(j',p''), (k, d, (h r))]
        offc2 = offc.reshape(KK, 2, NCH, NJH, 128)   # [k, d, h, j', p]
        offc2 = offc2.reshape(KK, 2, NCH, NJH, 8, 16)  # [k, d, h, j', r, p'']
        offgB_np = np.ascontiguousarray(
            offc2.transpose(3, 5, 0, 1, 2, 4).reshape(128, C288))
        in_maps.append({
            "img": imgs[b],
            "offg": offg_np,
            "grid": grids[half],
            "offgB": offgB_np,
            "gridB": gridsB[half],
            "wT": wT,
            "ident": ident,
            "dummyidx": dummyidx,
        })
    return in_maps


def _assemble(results):
    out = np.empty((B, COUT, HO, WO), np.float32)
    for core, r in enumerate(results):
        b, half = core // 2, core % 2
        # device column order per half: (r, j', p''); natural n = j'*128+r*16+p''
        o = r["out"].reshape(COUT, NCH, 8, NJH, 16)       # [c, h, r, j', p'']
        o = o.transpose(0, 1, 3, 2, 4).reshape(COUT, N)   # [c, (h j' r p'')]
        out[b, :, half * HOH:(half + 1) * HOH, :] = o.reshape(COUT, HOH, WO)
    return out


def kernel(x, offset, weight):
    from concourse.bass_utils import run_bass_kernel_spmd

    nc = _get_nc()
    in_maps = _prep_core_inputs(x, offset, weight)
    res = run_bass_kernel_spmd(nc, in_maps, core_ids=list(range(8)))
    return _assemble(res.results)
